# revision 12
# baseline (speedup 1.0000x reference)
"""Causal self-attention Trainium2 kernel (B=8, T=1024, C=768, H=12 heads).

Strategy: data-parallel over batch — one batch element per NeuronCore (8 cores).
Per core, everything is computed in a "transposed" layout so that no on-device
transposes are needed:

  qT, kT  [C, T]   = w_attn_{q,k}.T @ x.T          (x.T supplied by host)
  v_aug   [T, 781] = x @ [w_attn_v | 0]  (+ ones column per head, stride 65)
  sT_h    [Tk, Tq] = kT_h.T-slices @ qT_h          (keys on partitions)
  eT      = exp(sT / 8), causal mask via bf16 triangular multiply (DVE 4x mode)
  yT_aug  [65, Tq] = v_aug_h.T @ eT                (row 64 = softmax row-sums)
  yT_norm = yT * broadcast(1/sums)                 (broadcast via one-hot matmul)
  out     [T, C]   = yT_norm.T-slices @ w_proj

All matmuls run in bf16 (fp32 PSUM accumulation); bf16 halves DMA + SBUF
traffic and unlocks DVE 2x/4x modes. Work is organized in two phases by query
half (qc=0: queries 0-511, qc=1: 512-1023) so attention starts as soon as the
first half of QKT is ready, and softmax-normalization runs per head-pair so
the final projection is not gated on one big normalize.
"""
import sys

sys.path.insert(0, "/opt/trn_rl_repo")

import numpy as np
import ml_dtypes

import concourse.bass as bass
import concourse.bacc as bacc
import concourse.tile as tile
import concourse.mybir as mybir
from concourse.bass_utils import run_bass_kernel_spmd

f32 = mybir.dt.float32
f32r = mybir.dt.float32r
bf16 = mybir.dt.bfloat16
NPBF = ml_dtypes.bfloat16
EXP = mybir.ActivationFunctionType.Exp

B, T, C = 8, 1024, 768
H, D = 12, 64
DA = D + 1  # head stride in v_aug (extra ones column)
NK = C // 128  # 6 contraction tiles
NT = T // 128  # 8 token tiles
SCALE = 1.0 / np.sqrt(D)


def build():
    nc = bacc.Bacc("TRN2", target_bir_lowering=False, debug=False)
    xT = nc.dram_tensor("xT", [C, T], bf16, kind="ExternalInput")
    # 12 (q,k row-tile pairs: q0,k0,q1,k1,...) x contraction x 128
    wq = nc.dram_tensor("wq", [128, 2 * NK, NK * 128], bf16, kind="ExternalInput")
    wv = nc.dram_tensor("wv", [NK, 128, H * DA], bf16, kind="ExternalInput")
    wp = nc.dram_tensor("wp", [NK, 128, C], f32r, kind="ExternalInput")
    msk = nc.dram_tensor("msk", [128, 128], bf16, kind="ExternalInput")
    onesc = nc.dram_tensor("onesc", [128, H], bf16, kind="ExternalInput")
    sel2 = nc.dram_tensor("sel2", [2, 128], f32r, kind="ExternalInput")
    out = nc.dram_tensor("out", [T, C], f32, kind="ExternalOutput")

    with tile.TileContext(nc) as tc:
        with (
            tc.tile_pool(name="const", bufs=1) as const,
            tc.tile_pool(name="exp", bufs=4) as expp,
            tc.tile_pool(name="nrm", bufs=2) as nrm,
            tc.tile_pool(name="psc", bufs=3, space="PSUM") as psc,
            tc.tile_pool(name="psm", bufs=2, space="PSUM") as psm,
        ):
            # ---- resident SBUF tensors ----
            xTall = const.tile([128, NK * T], bf16, tag="xTall")
            xT_t = [xTall[:, i * T:(i + 1) * T] for i in range(NK)]
            wqall = const.tile([128, 2 * NK * NK * 128], bf16, tag="wqall")
            wqv = wqall.rearrange("p (m k c) -> p m k c", m=2 * NK, k=NK)
            wvall = const.tile([128, NK * H * DA], bf16, tag="wvall")
            wv_t = [wvall[:, i * H * DA:(i + 1) * H * DA] for i in range(NK)]
            wpall = const.tile([128, NK * C], f32r, tag="wpall")
            wp_t = [wpall[:, i * C:(i + 1) * C] for i in range(NK)]
            # per head-pair: [:, 0, :] = qT rows, [:, 1, :] = kT rows
            qkp_t = [const.tile([128, 2 * T], bf16, name=f"qkp{i}", tag=f"qkp{i}") for i in range(NK)]
            qkp_v = [qkp_t[i].rearrange("p (i n) -> p i n", i=2) for i in range(NK)]
            v_t = [const.tile([128, H * DA], bf16, name=f"vs{t}", tag=f"v{t}") for t in range(NT)]
            yT_t = [const.tile([128, T], f32r, name=f"yTs{i}", tag=f"yT{i}") for i in range(NK)]
            msk_t = const.tile([128, 128], bf16, tag="msk")
            ones_t = const.tile([128, H], bf16, tag="ones")
            sel2_t = const.tile([2, 128], f32r, tag="sel2")
            # row 0: even head of pair, row 1: odd head; column block hp*T+qs
            sums_t = const.tile([2, NK * T], f32, tag="sums")

            xTd = xT.rearrange("(i p) n -> p i n", p=128)
            xTv = xTall.rearrange("p (i n) -> p i n", n=T)
            wqd = wq.rearrange("p m n -> p m n")
            wqs = wqall.rearrange("p (m n) -> p m n", m=2 * NK)

            def qk_pair(hp, qc, evac=None):
                """q/k projections for head-pair hp, query block qc (512 wide)."""
                qs = slice(qc * 512, (qc + 1) * 512)
                ps = psc.tile([128, 1024], f32, tag="sc", name="psqk")
                for i, mi in enumerate((2 * hp, 2 * hp + 1)):
                    for kk in range(NK):
                        nc.tensor.matmul(
                            ps[:, i * 512:(i + 1) * 512],
                            wqv[:, mi, kk, :],
                            xT_t[kk][:, qs],
                            start=(kk == 0),
                            stop=(kk == NK - 1),
                        )
                dst = qkp_v[hp][:, :, qs]
                (evac or nc.vector.tensor_copy)(dst, ps.rearrange("p (i n) -> p i n", i=2))

            def v_tile(t, evac=None):
                ps = psc.tile([128, 1024], f32, tag="sc", name="psv")
                for n0, nw in ((0, 512), (512, H * DA - 512)):
                    for kk in range(NK):
                        nc.tensor.matmul(
                            ps[:, n0:n0 + nw],
                            xT_t[kk][:, t * 128:(t + 1) * 128],
                            wv_t[kk][:, n0:n0 + nw],
                            start=(kk == 0),
                            stop=(kk == NK - 1),
                        )
                vv = v_t[t].rearrange("p (h e) -> p h e", e=DA)[:, :, 0:D]
                pv = ps[:, :H * DA].rearrange("p (h e) -> p h e", e=DA)[:, :, 0:D]
                (evac or nc.vector.tensor_copy)(vv, pv)

            def attention(hp, qc):
                qs = slice(qc * 512, (qc + 1) * 512)
                nkt = 4 * (qc + 1)
                qT = qkp_v[hp][:, 0, :]
                kT = qkp_v[hp][:, 1, :]
                ypA = psm.tile([128, 512], f32, tag="mm", name="ypA")
                ypB = psm.tile([128, 512], f32, tag="mm", name="ypB")
                exs = {}
                # software pipeline: attv trails scores/exp by two kt
                LAG = 2
                for kt in range(nkt + LAG):
                    if kt < nkt:
                        ks = slice(kt * 128, (kt + 1) * 128)
                        pos = max(kt * 128 - qc * 512, 0)  # first visible column
                        qv = slice(qc * 512 + pos, (qc + 1) * 512)
                        sp = psc.tile([128, 1024], f32, tag="sc", name="sp")
                        nc.tensor.matmul(
                            sp[:, pos:512], kT[0:64, ks], qT[0:64, qv],
                            start=True, stop=True,
                        )
                        nc.tensor.matmul(
                            sp[:, 512 + pos:1024], kT[64:128, ks], qT[64:128, qv],
                            start=True, stop=True,
                        )
                        ex = expp.tile([128, 1024], bf16, tag="ex", bufs=4, name="ex")
                        if pos == 0:
                            nc.scalar.activation(ex, sp, EXP, scale=float(SCALE))
                        else:
                            exv = ex.rearrange("p (i n) -> p i n", i=2)[:, :, pos:512]
                            spv = sp.rearrange("p (i n) -> p i n", i=2)[:, :, pos:512]
                            nc.scalar.activation(exv, spv, EXP, scale=float(SCALE))
                        if kt * 128 >= qc * 512:  # diagonal tile: triangular mask
                            exd = ex.rearrange("p (i n) -> p i n", i=2)[:, :, pos:pos + 128]
                            nc.vector.tensor_mul(
                                exd, exd, msk_t[:, None, :].to_broadcast((128, 2, 128))
                            )
                        exs[kt] = (ex, pos)
                    if kt >= LAG:
                        pk = kt - LAG
                        exp_, ppos = exs.pop(pk)
                        for h, yp, half in ((2 * hp, ypA, 0), (2 * hp + 1, ypB, 1)):
                            nc.tensor.matmul(
                                yp[:DA, ppos:512],
                                v_t[pk][:, h * DA:(h + 1) * DA],
                                exp_[:, half * 512 + ppos:(half + 1) * 512],
                                start=(pk == 0), stop=(pk == nkt - 1),
                            )
                ss = slice(hp * T + qc * 512, hp * T + (qc + 1) * 512)
                for r, yp, off in ((0, ypA, 0), (1, ypB, 64)):
                    stage = expp.tile([DA, 512], f32r, tag="ystage", bufs=2, name="stage")
                    nc.vector.tensor_copy(stage, yp[:DA, :])
                    nc.sync.dma_start(out=yT_t[hp][off:off + 64, qs], in_=stage[:D, :])
                    nc.sync.dma_start(
                        out=sums_t[r:r + 1, ss], in_=stage[D:DA, :].bitcast(f32)
                    )

            def normalize(hp, qc):
                qs = slice(qc * 512, (qc + 1) * 512)
                ss = slice(hp * T + qc * 512, hp * T + (qc + 1) * 512)
                rec = nrm.tile([2, 512], f32r, tag="rec", name="rec")
                nc.vector.reciprocal_approx_fast(sums_t[:, ss], sums_t[:, ss])
                with nc.allow_low_precision(reason="f32r recip feeds f32r matmul"):
                    nc.vector.tensor_copy(rec, sums_t[:, ss])
                bc = psc.tile([128, 512], f32, tag="sc", name="bc")
                nc.tensor.matmul(bc, sel2_t, rec, start=True, stop=True)
                nc.vector.tensor_mul(yT_t[hp][:, qs], yT_t[hp][:, qs], bc.bitcast(f32r))

            def project(ts_range, evac=None):
                for t in ts_range:
                    pp = psc.tile([128, 1024], f32, tag="sc", name="pp")
                    for n0, nw in ((0, 512), (512, 256)):
                        for kk in range(NK):
                            nc.tensor.matmul(
                                pp[:, n0:n0 + nw],
                                yT_t[kk][:, t * 128:(t + 1) * 128],
                                wp_t[kk][:, n0:n0 + nw],
                                start=(kk == 0),
                                stop=(kk == NK - 1),
                            )
                    ostage = expp.tile([128, C], f32, tag="ostage", bufs=2, name="ostage")
                    (evac or nc.vector.tensor_copy)(ostage, pp[:, :C])
                    nc.sync.dma_start(out=out[t * 128:(t + 1) * 128, :], in_=ostage)

            # ---- DMA order: unblock phase A asap ----
            nc.sync.dma_start(out=ones_t, in_=onesc[:, :])
            nc.sync.dma_start(out=wqs[:, 0:2, :], in_=wqd[:, 0:2, :])
            for i in range(NK):
                nc.sync.dma_start(out=xT_t[i][:, 0:512], in_=xT[i * 128:(i + 1) * 128, 0:512])
            nc.sync.dma_start(
                out=wvall.rearrange("p (i n) -> p i n", i=NK),
                in_=wv.rearrange("i p n -> p i n"),
            )
            nc.sync.dma_start(out=wqs[:, 2:12, :], in_=wqd[:, 2:12, :])
            nc.sync.dma_start(out=xTv[:, :, 512:1024], in_=xTd[:, :, 512:1024])
            nc.sync.dma_start(
                out=wpall.rearrange("p (i n) -> p i n", i=NK),
                in_=wp.rearrange("i p n -> p i n"),
            )
            nc.sync.dma_start(out=msk_t, in_=msk[:, :])
            nc.sync.dma_start(out=sel2_t, in_=sel2[:, :])

            # ones columns of v tiles: written once, in the idle prologue
            for t in range(NT):
                ones_ap = v_t[t].rearrange("p (h e) -> p h e", e=DA)[:, :, D]
                nc.vector.tensor_copy(ones_ap, ones_t)

            # ---- phase A: query block 0 ----
            qk_pair(0, 0, evac=nc.scalar.copy)
            for t in range(4):
                v_tile(t, evac=nc.scalar.copy if t < 2 else None)
            qk_pair(1, 0, evac=nc.scalar.copy)
            for hp in range(NK):
                if hp + 2 < NK:
                    qk_pair(hp + 2, 0, evac=nc.scalar.copy)
                attention(hp, 0)
                normalize(hp, 0)

            # ---- phase B: query block 1 ----
            qk_pair(0, 1)
            v_tile(4, evac=nc.scalar.copy)
            v_tile(5)
            qk_pair(1, 1)
            v_tile(6, evac=nc.scalar.copy)
            v_tile(7)
            for hp in range(NK):
                if hp + 2 < NK:
                    qk_pair(hp + 2, 1)
                attention(hp, 1)
                normalize(hp, 1)
                if hp >= 2:
                    project([hp - 2])
            project(range(4, NT), evac=nc.scalar.copy)

    nc.compile()
    return nc


_nc = None


def _get_nc():
    global _nc
    if _nc is None:
        _nc = build()
    return _nc


def _host_prep(w_attn, w_proj):
    # q/k row-tiles, interleaved as (q0,k0,q1,k1,...) pairs, partition-major
    wq_m = w_attn[:, :2 * C].reshape(NK, 128, 2 * NK, 128).transpose(2, 1, 0, 3)
    order = [x for hp in range(NK) for x in (hp, NK + hp)]
    wqh = np.ascontiguousarray(
        wq_m[order].transpose(1, 0, 2, 3).reshape(128, 2 * NK, NK * 128)
    ).astype(NPBF)
    wv_aug = np.zeros((C, H, DA), np.float32)
    wv_aug[:, :, :D] = w_attn[:, 2 * C:].reshape(C, H, D)
    wv = np.ascontiguousarray(wv_aug.reshape(NK, 128, H * DA)).astype(NPBF)
    wp = np.ascontiguousarray(w_proj.reshape(NK, 128, C))
    msk = np.triu(np.ones((128, 128), np.float32)).astype(NPBF)
    onesc = np.ones((128, H), NPBF)
    sel2 = np.zeros((2, 128), np.float32)
    sel2[0, 0:64] = 1.0
    sel2[1, 64:128] = 1.0
    return wqh, wv, wp, msk, onesc, sel2


def kernel(x, w_attn, w_proj):
    x = np.asarray(x, dtype=np.float32)
    w_attn = np.asarray(w_attn, dtype=np.float32)
    w_proj = np.asarray(w_proj, dtype=np.float32)
    wqh, wv, wp, msk, onesc, sel2 = _host_prep(w_attn, w_proj)
    in_maps = [
        {
            "xT": np.ascontiguousarray(x[b].T).astype(NPBF),
            "wq": wqh,
            "wv": wv,
            "wp": wp,
            "msk": msk,
            "onesc": onesc,
            "sel2": sel2,
        }
        for b in range(B)
    ]
    last_err = None
    for _attempt in range(3):
        try:
            res = run_bass_kernel_spmd(_get_nc(), in_maps, list(range(B)))
            return np.stack([res.results[b]["out"] for b in range(B)], axis=0)
        except Exception as e:  # transient device wedge: retry
            last_err = e
    raise last_err


# revision 14
# speedup vs baseline: 1.5401x; 1.5401x over previous
"""Causal self-attention Trainium2 kernel (B=8, T=1024, C=768, H=12 heads).

Strategy: data-parallel over batch — one batch element per NeuronCore (8 cores).
Per core, everything is computed in a "transposed" layout so that no on-device
transposes are needed:

  qT, kT  [C, T]   = w_attn_{q,k}.T @ x.T          (x.T supplied by host)
  v_aug   [T, 781] = x @ [w_attn_v | 0]  (+ ones column per head, stride 65)
  sT_h    [Tk, Tq] = kT_h.T-slices @ qT_h          (keys on partitions)
  eT      = exp(sT / 8), causal mask via bf16 triangular multiply (DVE 4x mode)
  yT_aug  [65, Tq] = v_aug_h.T @ eT                (row 64 = softmax row-sums)
  yT_norm = yT * broadcast(1/sums)                 (broadcast via one-hot matmul)
  out     [T, C]   = yT_norm.T-slices @ w_proj

All matmuls run in bf16 (fp32 PSUM accumulation); bf16 halves DMA + SBUF
traffic and unlocks DVE 2x/4x modes. Work is organized in two phases by query
half (qc=0: queries 0-511, qc=1: 512-1023) so attention starts as soon as the
first half of QKT is ready, and softmax-normalization runs per head-pair so
the final projection is not gated on one big normalize.
"""
import sys

sys.path.insert(0, "/opt/trn_rl_repo")

import numpy as np
import ml_dtypes

import concourse.bass as bass
import concourse.bacc as bacc
import concourse.tile as tile
import concourse.mybir as mybir
from concourse.bass_utils import run_bass_kernel_spmd

f32 = mybir.dt.float32
f32r = mybir.dt.float32r
bf16 = mybir.dt.bfloat16
NPBF = ml_dtypes.bfloat16
EXP = mybir.ActivationFunctionType.Exp

B, T, C = 8, 1024, 768
H, D = 12, 64
DA = D + 1  # head stride in v_aug (extra ones column)
NK = C // 128  # 6 contraction tiles
NT = T // 128  # 8 token tiles
SCALE = 1.0 / np.sqrt(D)


def build():
    nc = bacc.Bacc("TRN2", target_bir_lowering=False, debug=False)
    xT = nc.dram_tensor("xT", [C, T], bf16, kind="ExternalInput")
    # 12 (q,k row-tile pairs: q0,k0,q1,k1,...) x contraction x 128
    wq = nc.dram_tensor("wq", [128, 2 * NK, NK * 128], bf16, kind="ExternalInput")
    wv = nc.dram_tensor("wv", [NK, 128, H * DA], bf16, kind="ExternalInput")
    wp = nc.dram_tensor("wp", [NK, 128, C], f32r, kind="ExternalInput")
    msk = nc.dram_tensor("msk", [128, 128], bf16, kind="ExternalInput")
    onesc = nc.dram_tensor("onesc", [128, H], bf16, kind="ExternalInput")
    sel2 = nc.dram_tensor("sel2", [2, 128], f32r, kind="ExternalInput")
    out = nc.dram_tensor("out", [T, C], f32, kind="ExternalOutput")

    with tile.TileContext(nc) as tc:
        with (
            tc.tile_pool(name="const", bufs=1) as const,
            tc.tile_pool(name="exp", bufs=4) as expp,
            tc.tile_pool(name="nrm", bufs=2) as nrm,
            tc.tile_pool(name="psc", bufs=3, space="PSUM") as psc,
            tc.tile_pool(name="psm", bufs=2, space="PSUM") as psm,
        ):
            # ---- resident SBUF tensors ----
            xTall = const.tile([128, NK * T], bf16, tag="xTall")
            xT_t = [xTall[:, i * T:(i + 1) * T] for i in range(NK)]
            wqall = const.tile([128, 2 * NK * NK * 128], bf16, tag="wqall")
            wqv = wqall.rearrange("p (m k c) -> p m k c", m=2 * NK, k=NK)
            wvall = const.tile([128, NK * H * DA], bf16, tag="wvall")
            wv_t = [wvall[:, i * H * DA:(i + 1) * H * DA] for i in range(NK)]
            wpall = const.tile([128, NK * C], f32r, tag="wpall")
            wp_t = [wpall[:, i * C:(i + 1) * C] for i in range(NK)]
            # per head-pair: [:, 0, :] = qT rows, [:, 1, :] = kT rows
            qkp_t = [const.tile([128, 2 * T], bf16, name=f"qkp{i}", tag=f"qkp{i}") for i in range(NK)]
            qkp_v = [qkp_t[i].rearrange("p (i n) -> p i n", i=2) for i in range(NK)]
            v_t = [const.tile([128, H * DA], bf16, name=f"vs{t}", tag=f"v{t}") for t in range(NT)]
            yT_t = [const.tile([128, T], f32r, name=f"yTs{i}", tag=f"yT{i}") for i in range(NK)]
            msk_t = const.tile([128, 128], bf16, tag="msk")
            ones_t = const.tile([128, H], bf16, tag="ones")
            sel2_t = const.tile([2, 128], f32r, tag="sel2")
            # row 0: even head of pair, row 1: odd head; column block hp*T+qs
            sums_t = const.tile([2, NK * T], f32, tag="sums")

            xTd = xT.rearrange("(i p) n -> p i n", p=128)
            xTv = xTall.rearrange("p (i n) -> p i n", n=T)
            wqd = wq.rearrange("p m n -> p m n")
            wqs = wqall.rearrange("p (m n) -> p m n", m=2 * NK)

            def qk_pair(hp, qc, evac=None):
                """q/k projections for head-pair hp, query block qc (512 wide)."""
                qs = slice(qc * 512, (qc + 1) * 512)
                ps = psc.tile([128, 1024], f32, tag="sc", name="psqk")
                for i, mi in enumerate((2 * hp, 2 * hp + 1)):
                    for kk in range(NK):
                        nc.tensor.matmul(
                            ps[:, i * 512:(i + 1) * 512],
                            wqv[:, mi, kk, :],
                            xT_t[kk][:, qs],
                            start=(kk == 0),
                            stop=(kk == NK - 1),
                        )
                dst = qkp_v[hp][:, :, qs]
                (evac or nc.vector.tensor_copy)(dst, ps.rearrange("p (i n) -> p i n", i=2))

            def v_tile(t, evac=None):
                ps = psc.tile([128, 1024], f32, tag="sc", name="psv")
                for n0, nw in ((0, 512), (512, H * DA - 512)):
                    for kk in range(NK):
                        nc.tensor.matmul(
                            ps[:, n0:n0 + nw],
                            xT_t[kk][:, t * 128:(t + 1) * 128],
                            wv_t[kk][:, n0:n0 + nw],
                            start=(kk == 0),
                            stop=(kk == NK - 1),
                        )
                vv = v_t[t].rearrange("p (h e) -> p h e", e=DA)[:, :, 0:D]
                pv = ps[:, :H * DA].rearrange("p (h e) -> p h e", e=DA)[:, :, 0:D]
                (evac or nc.vector.tensor_copy)(vv, pv)

            def attention(hp, qc):
                qs = slice(qc * 512, (qc + 1) * 512)
                nkt = 4 * (qc + 1)
                qT = qkp_v[hp][:, 0, :]
                kT = qkp_v[hp][:, 1, :]
                ypA = psm.tile([128, 512], f32, tag="mm", name="ypA")
                ypB = psm.tile([128, 512], f32, tag="mm", name="ypB")
                exs = {}
                # software pipeline: attv trails scores/exp by two kt
                LAG = 2
                for kt in range(nkt + LAG):
                    if kt < nkt:
                        ks = slice(kt * 128, (kt + 1) * 128)
                        pos = max(kt * 128 - qc * 512, 0)  # first visible column
                        qv = slice(qc * 512 + pos, (qc + 1) * 512)
                        sp = psc.tile([128, 1024], f32, tag="sc", name="sp")
                        nc.tensor.matmul(
                            sp[:, pos:512], kT[0:64, ks], qT[0:64, qv],
                            start=True, stop=True,
                        )
                        nc.tensor.matmul(
                            sp[:, 512 + pos:1024], kT[64:128, ks], qT[64:128, qv],
                            start=True, stop=True,
                        )
                        ex = expp.tile([128, 1024], bf16, tag="ex", bufs=4, name="ex")
                        if pos == 0:
                            nc.scalar.activation(ex, sp, EXP, scale=float(SCALE))
                        else:
                            exv = ex.rearrange("p (i n) -> p i n", i=2)[:, :, pos:512]
                            spv = sp.rearrange("p (i n) -> p i n", i=2)[:, :, pos:512]
                            nc.scalar.activation(exv, spv, EXP, scale=float(SCALE))
                        if kt * 128 >= qc * 512:  # diagonal tile: triangular mask
                            exd = ex.rearrange("p (i n) -> p i n", i=2)[:, :, pos:pos + 128]
                            nc.vector.tensor_mul(
                                exd, exd, msk_t[:, None, :].to_broadcast((128, 2, 128))
                            )
                        exs[kt] = (ex, pos)
                    if kt >= LAG:
                        pk = kt - LAG
                        exp_, ppos = exs.pop(pk)
                        for h, yp, half in ((2 * hp, ypA, 0), (2 * hp + 1, ypB, 1)):
                            nc.tensor.matmul(
                                yp[:DA, ppos:512],
                                v_t[pk][:, h * DA:(h + 1) * DA],
                                exp_[:, half * 512 + ppos:(half + 1) * 512],
                                start=(pk == 0), stop=(pk == nkt - 1),
                            )
                ss = slice(hp * T + qc * 512, hp * T + (qc + 1) * 512)
                for r, yp, off in ((0, ypA, 0), (1, ypB, 64)):
                    stage = expp.tile([DA, 512], f32r, tag="ystage", bufs=2, name="stage")
                    nc.vector.tensor_copy(stage, yp[:DA, :])
                    nc.sync.dma_start(out=yT_t[hp][off:off + 64, qs], in_=stage[:D, :])
                    nc.sync.dma_start(
                        out=sums_t[r:r + 1, ss], in_=stage[D:DA, :].bitcast(f32)
                    )

            def normalize(hp, qc):
                qs = slice(qc * 512, (qc + 1) * 512)
                ss = slice(hp * T + qc * 512, hp * T + (qc + 1) * 512)
                rec = nrm.tile([2, 512], f32r, tag="rec", name="rec")
                nc.vector.reciprocal_approx_fast(sums_t[:, ss], sums_t[:, ss])
                with nc.allow_low_precision(reason="f32r recip feeds f32r matmul"):
                    nc.vector.tensor_copy(rec, sums_t[:, ss])
                bc = psc.tile([128, 512], f32, tag="sc", name="bc")
                nc.tensor.matmul(bc, sel2_t, rec, start=True, stop=True)
                nc.vector.tensor_mul(yT_t[hp][:, qs], yT_t[hp][:, qs], bc.bitcast(f32r))

            def project(ts_range, evac=None):
                for t in ts_range:
                    pp = psc.tile([128, 1024], f32, tag="sc", name="pp")
                    for n0, nw in ((0, 512), (512, 256)):
                        for kk in range(NK):
                            nc.tensor.matmul(
                                pp[:, n0:n0 + nw],
                                yT_t[kk][:, t * 128:(t + 1) * 128],
                                wp_t[kk][:, n0:n0 + nw],
                                start=(kk == 0),
                                stop=(kk == NK - 1),
                            )
                    ostage = expp.tile([128, C], f32, tag="ostage", bufs=2, name="ostage")
                    (evac or nc.vector.tensor_copy)(ostage, pp[:, :C])
                    nc.sync.dma_start(out=out[t * 128:(t + 1) * 128, :], in_=ostage)

            # ---- DMA order: unblock phase A asap ----
            nc.sync.dma_start(out=ones_t, in_=onesc[:, :])
            nc.sync.dma_start(out=wqs[:, 0:2, :], in_=wqd[:, 0:2, :])
            for i in range(NK):
                nc.sync.dma_start(out=xT_t[i][:, 0:512], in_=xT[i * 128:(i + 1) * 128, 0:512])
            nc.sync.dma_start(
                out=wvall.rearrange("p (i n) -> p i n", i=NK),
                in_=wv.rearrange("i p n -> p i n"),
            )
            nc.sync.dma_start(out=wqs[:, 2:12, :], in_=wqd[:, 2:12, :])
            nc.sync.dma_start(out=xTv[:, :, 512:1024], in_=xTd[:, :, 512:1024])
            nc.sync.dma_start(
                out=wpall.rearrange("p (i n) -> p i n", i=NK),
                in_=wp.rearrange("i p n -> p i n"),
            )
            nc.sync.dma_start(out=msk_t, in_=msk[:, :])
            nc.sync.dma_start(out=sel2_t, in_=sel2[:, :])

            # ones columns of v tiles: written once, in the idle prologue
            for t in range(NT):
                ones_ap = v_t[t].rearrange("p (h e) -> p h e", e=DA)[:, :, D]
                nc.vector.tensor_copy(ones_ap, ones_t)

            # ---- phase A: query block 0 ----
            # normalize lags attention by 2 head-pairs so its broadcast matmul
            # never stalls the tensor queue on the sums->reciprocal chain
            qk_pair(0, 0, evac=nc.scalar.copy)
            qk_pair(1, 0, evac=nc.scalar.copy)
            v_tile(0, evac=nc.scalar.copy)
            v_tile(1, evac=nc.scalar.copy)
            qk_pair(2, 0, evac=nc.scalar.copy)
            v_tile(2)
            v_tile(3)
            for hp in range(NK):
                if hp + 3 < NK:
                    qk_pair(hp + 3, 0, evac=nc.scalar.copy)
                attention(hp, 0)
                if hp >= 2:
                    normalize(hp - 2, 0)

            # ---- phase B: query block 1 (finishing phase A normalizes) ----
            qk_pair(0, 1)
            v_tile(4)
            v_tile(5)
            qk_pair(1, 1)
            v_tile(6)
            normalize(4, 0)
            v_tile(7)
            qk_pair(2, 1)
            normalize(5, 0)
            for hp in range(NK):
                if hp + 3 < NK:
                    qk_pair(hp + 3, 1)
                attention(hp, 1)
                if hp >= 2:
                    normalize(hp - 2, 1)
                if hp >= 4:
                    project([hp - 4])
            normalize(4, 1)
            project([2])
            project([3])
            normalize(5, 1)
            project(range(4, NT), evac=nc.scalar.copy)

    nc.compile()
    return nc


_nc = None


def _get_nc():
    global _nc
    if _nc is None:
        _nc = build()
    return _nc


def _host_prep(w_attn, w_proj):
    # q/k row-tiles, interleaved as (q0,k0,q1,k1,...) pairs, partition-major
    wq_m = w_attn[:, :2 * C].reshape(NK, 128, 2 * NK, 128).transpose(2, 1, 0, 3)
    order = [x for hp in range(NK) for x in (hp, NK + hp)]
    wqh = np.ascontiguousarray(
        wq_m[order].transpose(1, 0, 2, 3).reshape(128, 2 * NK, NK * 128)
    ).astype(NPBF)
    wv_aug = np.zeros((C, H, DA), np.float32)
    wv_aug[:, :, :D] = w_attn[:, 2 * C:].reshape(C, H, D)
    wv = np.ascontiguousarray(wv_aug.reshape(NK, 128, H * DA)).astype(NPBF)
    wp = np.ascontiguousarray(w_proj.reshape(NK, 128, C))
    msk = np.triu(np.ones((128, 128), np.float32)).astype(NPBF)
    onesc = np.ones((128, H), NPBF)
    sel2 = np.zeros((2, 128), np.float32)
    sel2[0, 0:64] = 1.0
    sel2[1, 64:128] = 1.0
    return wqh, wv, wp, msk, onesc, sel2


def kernel(x, w_attn, w_proj):
    x = np.asarray(x, dtype=np.float32)
    w_attn = np.asarray(w_attn, dtype=np.float32)
    w_proj = np.asarray(w_proj, dtype=np.float32)
    wqh, wv, wp, msk, onesc, sel2 = _host_prep(w_attn, w_proj)
    in_maps = [
        {
            "xT": np.ascontiguousarray(x[b].T).astype(NPBF),
            "wq": wqh,
            "wv": wv,
            "wp": wp,
            "msk": msk,
            "onesc": onesc,
            "sel2": sel2,
        }
        for b in range(B)
    ]
    last_err = None
    for _attempt in range(3):
        try:
            res = run_bass_kernel_spmd(_get_nc(), in_maps, list(range(B)))
            return np.stack([res.results[b]["out"] for b in range(B)], axis=0)
        except Exception as e:  # transient device wedge: retry
            last_err = e
    raise last_err


# revision 15
# speedup vs baseline: 1.6688x; 1.0836x over previous
"""Causal self-attention Trainium2 kernel (B=8, T=1024, C=768, H=12 heads).

Strategy: data-parallel over batch — one batch element per NeuronCore (8 cores).
Per core, everything is computed in a "transposed" layout so that no on-device
transposes are needed:

  qT, kT  [C, T]   = w_attn_{q,k}.T @ x.T          (x.T supplied by host)
  v_aug   [T, 781] = x @ [w_attn_v | 0]  (+ ones column per head, stride 65)
  sT_h    [Tk, Tq] = kT_h.T-slices @ qT_h          (keys on partitions)
  eT      = exp(sT / 8), causal mask via bf16 triangular multiply (DVE 4x mode)
  yT_aug  [65, Tq] = v_aug_h.T @ eT                (row 64 = softmax row-sums)
  yT_norm = yT * broadcast(1/sums)                 (broadcast via one-hot matmul)
  out     [T, C]   = yT_norm.T-slices @ w_proj

All matmuls run in bf16 (fp32 PSUM accumulation); bf16 halves DMA + SBUF
traffic and unlocks DVE 2x/4x modes. Work is organized in two phases by query
half (qc=0: queries 0-511, qc=1: 512-1023) so attention starts as soon as the
first half of QKT is ready, and softmax-normalization runs per head-pair so
the final projection is not gated on one big normalize.
"""
import sys

sys.path.insert(0, "/opt/trn_rl_repo")

import numpy as np
import ml_dtypes

import concourse.bass as bass
import concourse.bacc as bacc
import concourse.tile as tile
import concourse.mybir as mybir
from concourse.bass_utils import run_bass_kernel_spmd

f32 = mybir.dt.float32
f32r = mybir.dt.float32r
bf16 = mybir.dt.bfloat16
NPBF = ml_dtypes.bfloat16
EXP = mybir.ActivationFunctionType.Exp

B, T, C = 8, 1024, 768
H, D = 12, 64
DA = D + 1  # head stride in v_aug (extra ones column)
NK = C // 128  # 6 contraction tiles
NT = T // 128  # 8 token tiles
SCALE = 1.0 / np.sqrt(D)


def build():
    nc = bacc.Bacc("TRN2", target_bir_lowering=False, debug=False)
    xT = nc.dram_tensor("xT", [C, T], bf16, kind="ExternalInput")
    # 12 (q,k row-tile pairs: q0,k0,q1,k1,...) x contraction x 128
    wq = nc.dram_tensor("wq", [128, 2 * NK, NK * 128], bf16, kind="ExternalInput")
    wv = nc.dram_tensor("wv", [NK, 128, H * DA], bf16, kind="ExternalInput")
    wp = nc.dram_tensor("wp", [NK, 128, C], f32r, kind="ExternalInput")
    msk = nc.dram_tensor("msk", [128, 128], bf16, kind="ExternalInput")
    onesc = nc.dram_tensor("onesc", [128, H], bf16, kind="ExternalInput")
    sel2 = nc.dram_tensor("sel2", [2, 128], f32r, kind="ExternalInput")
    out = nc.dram_tensor("out", [T, C], f32, kind="ExternalOutput")

    with tile.TileContext(nc) as tc:
        with (
            tc.tile_pool(name="const", bufs=1) as const,
            tc.tile_pool(name="exp", bufs=4) as expp,
            tc.tile_pool(name="nrm", bufs=2) as nrm,
            tc.tile_pool(name="psc", bufs=3, space="PSUM") as psc,
            tc.tile_pool(name="psm", bufs=1, space="PSUM") as psm,
        ):
            # ---- resident SBUF tensors ----
            xTall = const.tile([128, NK * T], bf16, tag="xTall")
            xT_t = [xTall[:, i * T:(i + 1) * T] for i in range(NK)]
            wqall = const.tile([128, 2 * NK * NK * 128], bf16, tag="wqall")
            wqv = wqall.rearrange("p (m k c) -> p m k c", m=2 * NK, k=NK)
            wvall = const.tile([128, NK * H * DA], bf16, tag="wvall")
            wv_t = [wvall[:, i * H * DA:(i + 1) * H * DA] for i in range(NK)]
            wpall = const.tile([128, NK * C], f32r, tag="wpall")
            wp_t = [wpall[:, i * C:(i + 1) * C] for i in range(NK)]
            # per head-pair: [:, 0, :] = qT rows, [:, 1, :] = kT rows
            qkp_t = [const.tile([128, 2 * T], bf16, name=f"qkp{i}", tag=f"qkp{i}") for i in range(NK)]
            qkp_v = [qkp_t[i].rearrange("p (i n) -> p i n", i=2) for i in range(NK)]
            v_t = [const.tile([128, H * DA], bf16, name=f"vs{t}", tag=f"v{t}") for t in range(NT)]
            yT_t = [const.tile([128, T], f32r, name=f"yTs{i}", tag=f"yT{i}") for i in range(NK)]
            msk_t = const.tile([128, 128], bf16, tag="msk")
            ones_t = const.tile([128, H], bf16, tag="ones")
            sel2_t = const.tile([2, 128], f32r, tag="sel2")
            # row 0: even head of pair, row 1: odd head; column block hp*T+qs
            sums_t = const.tile([2, NK * T], f32, tag="sums")

            xTd = xT.rearrange("(i p) n -> p i n", p=128)
            xTv = xTall.rearrange("p (i n) -> p i n", n=T)
            wqd = wq.rearrange("p m n -> p m n")
            wqs = wqall.rearrange("p (m n) -> p m n", m=2 * NK)

            def qk_pair(hp, qc, evac=None):
                """q/k projections for head-pair hp, query block qc (512 wide)."""
                qs = slice(qc * 512, (qc + 1) * 512)
                ps = psc.tile([128, 1024], f32, tag="sc", name="psqk")
                for i, mi in enumerate((2 * hp, 2 * hp + 1)):
                    for kk in range(NK):
                        nc.tensor.matmul(
                            ps[:, i * 512:(i + 1) * 512],
                            wqv[:, mi, kk, :],
                            xT_t[kk][:, qs],
                            start=(kk == 0),
                            stop=(kk == NK - 1),
                        )
                dst = qkp_v[hp][:, :, qs]
                (evac or nc.vector.tensor_copy)(dst, ps.rearrange("p (i n) -> p i n", i=2))

            def v_tile(t, evac=None):
                ps = psc.tile([128, 1024], f32, tag="sc", name="psv")
                for n0, nw in ((0, 512), (512, H * DA - 512)):
                    for kk in range(NK):
                        nc.tensor.matmul(
                            ps[:, n0:n0 + nw],
                            xT_t[kk][:, t * 128:(t + 1) * 128],
                            wv_t[kk][:, n0:n0 + nw],
                            start=(kk == 0),
                            stop=(kk == NK - 1),
                        )
                vv = v_t[t].rearrange("p (h e) -> p h e", e=DA)[:, :, 0:D]
                pv = ps[:, :H * DA].rearrange("p (h e) -> p h e", e=DA)[:, :, 0:D]
                (evac or nc.vector.tensor_copy)(vv, pv)

            def attention(hp, qc):
                qs = slice(qc * 512, (qc + 1) * 512)
                nkt = 4 * (qc + 1)
                qT = qkp_v[hp][:, 0, :]
                kT = qkp_v[hp][:, 1, :]
                ypAB = psm.tile([128, 1024], f32, tag="mm", name="ypAB")
                ypA = ypAB[:, 0:512]
                ypB = ypAB[:, 512:1024]
                exs = {}
                # software pipeline: attv trails scores/exp (deeper for the
                # longer qc=1 phase where the scalar engine paces the loop)
                LAG = 2 + qc
                for kt in range(nkt + LAG):
                    if kt < nkt:
                        ks = slice(kt * 128, (kt + 1) * 128)
                        pos = max(kt * 128 - qc * 512, 0)  # first visible column
                        qv = slice(qc * 512 + pos, (qc + 1) * 512)
                        sp = psc.tile([128, 1024], f32, tag="sc", name="sp")
                        nc.tensor.matmul(
                            sp[:, pos:512], kT[0:64, ks], qT[0:64, qv],
                            start=True, stop=True,
                        )
                        nc.tensor.matmul(
                            sp[:, 512 + pos:1024], kT[64:128, ks], qT[64:128, qv],
                            start=True, stop=True,
                        )
                        ex = expp.tile([128, 1024], bf16, tag="ex", bufs=4, name="ex")
                        if pos == 0:
                            nc.scalar.activation(ex, sp, EXP, scale=float(SCALE))
                        else:
                            exv = ex.rearrange("p (i n) -> p i n", i=2)[:, :, pos:512]
                            spv = sp.rearrange("p (i n) -> p i n", i=2)[:, :, pos:512]
                            nc.scalar.activation(exv, spv, EXP, scale=float(SCALE))
                        if kt * 128 >= qc * 512:  # diagonal tile: triangular mask
                            exd = ex.rearrange("p (i n) -> p i n", i=2)[:, :, pos:pos + 128]
                            nc.vector.tensor_mul(
                                exd, exd, msk_t[:, None, :].to_broadcast((128, 2, 128))
                            )
                        exs[kt] = (ex, pos)
                    if kt >= LAG:
                        pk = kt - LAG
                        exp_, ppos = exs.pop(pk)
                        for h, yp, half in ((2 * hp, ypA, 0), (2 * hp + 1, ypB, 1)):
                            nc.tensor.matmul(
                                yp[:DA, ppos:512],
                                v_t[pk][:, h * DA:(h + 1) * DA],
                                exp_[:, half * 512 + ppos:(half + 1) * 512],
                                start=(pk == 0), stop=(pk == nkt - 1),
                            )
                ss = slice(hp * T + qc * 512, hp * T + (qc + 1) * 512)
                stage = expp.tile([DA, 1024], f32r, tag="ystage", bufs=2, name="stage")
                nc.vector.tensor_copy(stage, ypAB[:DA, :])
                nc.sync.dma_start(out=yT_t[hp][0:64, qs], in_=stage[:D, 0:512])
                nc.sync.dma_start(out=yT_t[hp][64:128, qs], in_=stage[:D, 512:1024])
                nc.sync.dma_start(out=sums_t[:, ss], in_=stage[D:DA, :].bitcast(f32))

            def normalize(hp, qc):
                qs = slice(qc * 512, (qc + 1) * 512)
                ss = slice(hp * T + qc * 512, hp * T + (qc + 1) * 512)
                rec = nrm.tile([2, 512], f32r, tag="rec", name="rec")
                nc.vector.reciprocal_approx_fast(sums_t[:, ss], sums_t[:, ss])
                with nc.allow_low_precision(reason="f32r recip feeds f32r matmul"):
                    nc.vector.tensor_copy(rec, sums_t[:, ss])
                bc = psc.tile([128, 512], f32, tag="sc", name="bc")
                nc.tensor.matmul(bc, sel2_t, rec, start=True, stop=True)
                nc.vector.tensor_mul(yT_t[hp][:, qs], yT_t[hp][:, qs], bc.bitcast(f32r))

            def project(ts_range, evac=None, split=False):
                for t in ts_range:
                    pp = psc.tile([128, 1024], f32, tag="sc", name="pp")
                    for n0, nw in ((0, 512), (512, 256)):
                        for kk in range(NK):
                            nc.tensor.matmul(
                                pp[:, n0:n0 + nw],
                                yT_t[kk][:, t * 128:(t + 1) * 128],
                                wp_t[kk][:, n0:n0 + nw],
                                start=(kk == 0),
                                stop=(kk == NK - 1),
                            )
                    ostage = expp.tile([128, C], f32, tag="ostage", bufs=2, name="ostage")
                    if split:  # final tile: halve the drain latency
                        nc.scalar.copy(ostage[:, 0:384], pp[:, 0:384])
                        nc.vector.tensor_copy(ostage[:, 384:C], pp[:, 384:C])
                        nc.sync.dma_start(
                            out=out[t * 128:(t + 1) * 128, 0:384], in_=ostage[:, 0:384]
                        )
                        nc.sync.dma_start(
                            out=out[t * 128:(t + 1) * 128, 384:C], in_=ostage[:, 384:C]
                        )
                    else:
                        (evac or nc.vector.tensor_copy)(ostage, pp[:, :C])
                        nc.sync.dma_start(out=out[t * 128:(t + 1) * 128, :], in_=ostage)

            # ---- DMA order: unblock phase A asap ----
            nc.sync.dma_start(out=ones_t, in_=onesc[:, :])
            nc.sync.dma_start(out=wqs[:, 0:1, :], in_=wqd[:, 0:1, :])
            nc.sync.dma_start(out=wqs[:, 1:2, :], in_=wqd[:, 1:2, :])
            for i in range(NK):
                nc.sync.dma_start(out=xT_t[i][:, 0:512], in_=xT[i * 128:(i + 1) * 128, 0:512])
            nc.sync.dma_start(out=wqs[:, 2:4, :], in_=wqd[:, 2:4, :])
            nc.sync.dma_start(
                out=wvall.rearrange("p (i n) -> p i n", i=NK),
                in_=wv.rearrange("i p n -> p i n"),
            )
            nc.sync.dma_start(out=wqs[:, 4:6, :], in_=wqd[:, 4:6, :])
            nc.sync.dma_start(out=msk_t, in_=msk[:, :])
            nc.sync.dma_start(out=sel2_t, in_=sel2[:, :])
            nc.sync.dma_start(out=wqs[:, 6:12, :], in_=wqd[:, 6:12, :])
            nc.sync.dma_start(out=xTv[:, :, 512:1024], in_=xTd[:, :, 512:1024])
            nc.sync.dma_start(
                out=wpall.rearrange("p (i n) -> p i n", i=NK),
                in_=wp.rearrange("i p n -> p i n"),
            )

            # ones columns of v tiles: written once, in the idle prologue
            for t in range(NT):
                ones_ap = v_t[t].rearrange("p (h e) -> p h e", e=DA)[:, :, D]
                nc.vector.tensor_copy(ones_ap, ones_t)

            # ---- phase A: query block 0 ----
            # normalize lags attention by 2 head-pairs so its broadcast matmul
            # never stalls the tensor queue on the sums->reciprocal chain
            qk_pair(0, 0, evac=nc.scalar.copy)
            qk_pair(1, 0, evac=nc.scalar.copy)
            v_tile(0, evac=nc.scalar.copy)
            v_tile(1, evac=nc.scalar.copy)
            qk_pair(2, 0, evac=nc.scalar.copy)
            v_tile(2)
            v_tile(3)
            for hp in range(NK):
                if hp + 3 < NK:
                    qk_pair(hp + 3, 0)
                attention(hp, 0)
                if hp >= 2:
                    normalize(hp - 2, 0)

            # ---- phase B: query block 1 (finishing phase A normalizes) ----
            qk_pair(0, 1)
            v_tile(4)
            v_tile(5)
            qk_pair(1, 1)
            v_tile(6)
            normalize(4, 0)
            v_tile(7)
            qk_pair(2, 1)
            normalize(5, 0)
            for hp in range(NK):
                if hp + 3 < NK:
                    qk_pair(hp + 3, 1)
                attention(hp, 1)
                if hp >= 2:
                    normalize(hp - 2, 1)
                if hp >= 4:
                    project([hp - 4])
            normalize(4, 1)
            project([2])
            project([3])
            normalize(5, 1)
            project([4], evac=nc.scalar.copy)
            project([5])
            project([6], evac=nc.scalar.copy)
            project([7], split=True)

    nc.compile()
    return nc


_nc = None


def _get_nc():
    global _nc
    if _nc is None:
        _nc = build()
    return _nc


def _host_prep(w_attn, w_proj):
    # q/k row-tiles, interleaved as (q0,k0,q1,k1,...) pairs, partition-major
    wq_m = w_attn[:, :2 * C].reshape(NK, 128, 2 * NK, 128).transpose(2, 1, 0, 3)
    order = [x for hp in range(NK) for x in (hp, NK + hp)]
    wqh = np.ascontiguousarray(
        wq_m[order].transpose(1, 0, 2, 3).reshape(128, 2 * NK, NK * 128)
    ).astype(NPBF)
    wv_aug = np.zeros((C, H, DA), np.float32)
    wv_aug[:, :, :D] = w_attn[:, 2 * C:].reshape(C, H, D)
    wv = np.ascontiguousarray(wv_aug.reshape(NK, 128, H * DA)).astype(NPBF)
    wp = np.ascontiguousarray(w_proj.reshape(NK, 128, C))
    msk = np.triu(np.ones((128, 128), np.float32)).astype(NPBF)
    onesc = np.ones((128, H), NPBF)
    sel2 = np.zeros((2, 128), np.float32)
    sel2[0, 0:64] = 1.0
    sel2[1, 64:128] = 1.0
    return wqh, wv, wp, msk, onesc, sel2


def kernel(x, w_attn, w_proj):
    x = np.asarray(x, dtype=np.float32)
    w_attn = np.asarray(w_attn, dtype=np.float32)
    w_proj = np.asarray(w_proj, dtype=np.float32)
    wqh, wv, wp, msk, onesc, sel2 = _host_prep(w_attn, w_proj)
    in_maps = [
        {
            "xT": np.ascontiguousarray(x[b].T).astype(NPBF),
            "wq": wqh,
            "wv": wv,
            "wp": wp,
            "msk": msk,
            "onesc": onesc,
            "sel2": sel2,
        }
        for b in range(B)
    ]
    last_err = None
    for _attempt in range(3):
        try:
            res = run_bass_kernel_spmd(_get_nc(), in_maps, list(range(B)))
            return np.stack([res.results[b]["out"] for b in range(B)], axis=0)
        except Exception as e:  # transient device wedge: retry
            last_err = e
    raise last_err
